# revision 65
# baseline (speedup 1.0000x reference)
"""Trainium2 Bass kernel for the DTVIN (dynamic-transition VIN) model.

Self-contained: accepts FULL inputs (batch 128), shards batch over 8
NeuronCores (16 samples/core, pure data parallel), runs one Bass program
per core via run_bass_kernel_spmd, returns full [128, 8, 49, 49] logits.

Per-core program (all fp16 compute, fp32 PSUM accumulation):
  conv1 (K=18 im2col matmul) -> relu -> conv2 (dj-folded im2col, K=450 x
  3 row-shifts) -> relu -> reward/trans conv (pad=2, 51x51, K=450 x 3) ->
  exp on drain -> softmax (tree-sum + reciprocal) -> 30-step value
  iteration on the Vector engine with (action,sample) partition layout ->
  per-pixel MLP (8->150->8) -> logits.
"""
import sys
import types
import numpy as np

for p in ("/opt/trn_rl_repo", "/root/.axon_site/_ro/trn_rl_repo"):
    if p not in sys.path:
        sys.path.append(p)

import concourse.bass as bass
import concourse.tile as tile
from concourse import mybir
from concourse.bass_utils import run_bass_kernel_spmd

FP16 = mybir.dt.float16
FP32 = mybir.dt.float32
AF = mybir.ActivationFunctionType

# geometry
S = 16          # samples per core
HID = 150
A = 8
G = 49          # conv grid
HP = 51         # value-iteration grid
KST = 30        # value-iteration steps
CH, CW = 53, 54          # big canvas (rows, cols) for conv-side tensors
PH, PW = 51, 51          # plane canvas for sT / scan tensors
CFLAT = CH * CW          # 2862
PFLAT = PH * PW          # 2652
# scan block layout: partition p = s*8 + rb; each partition owns 7 rows
RB = 8                   # row blocks per sample
RPB = 7                  # rows per block (8*7 = 56 >= 51; rows 51..55 zero)
VC = 53                  # V cols: 1 + 51 + 1 zero pads
RC = RPB * PW            # 357 elements per (row-block, col) plane


def _split_multi_waits(nc):
    """This walrus build accepts at most ONE sync wait per instruction.
    Move extra waits onto preceding same-engine NoOps (queues run in
    order, so nop-waits followed by the inst == waiting on all)."""
    n = 0
    for f in nc.m.functions:
        for bb in f.blocks:
            new_insts = []
            for ins in bb.instructions:
                si = ins.sync_info
                if si is not None and len(si.on_wait) > 1:
                    waits = list(si.on_wait)
                    for w in waits[:-1]:
                        n += 1
                        nop = mybir.InstNoOp(name=f"waitsplit_{n}", ins=[], outs=[])
                        nop.engine = ins.engine
                        nop.sync_info = mybir.SyncInfo(on_wait=[w], on_update=[])
                        new_insts.append(nop)
                    ins.sync_info = mybir.SyncInfo(on_wait=[waits[-1]],
                                                   on_update=list(si.on_update))
                new_insts.append(ins)
            bb.instructions = new_insts


def _part_ap(t, part_start, part_stride, part_num, free_ap, extra_off=0):
    """AP over tile `t` with a strided partition dim (for DMA only)."""
    full = t[:, :] if len(t.shape) == 2 else t[:, :, :] if len(t.shape) == 3 else t[:, :, :, :]
    pstride = full.ap[0][0]  # elements per partition step
    return bass.AP(tensor=full.tensor,
                   offset=full.offset + part_start * pstride + extra_off,
                   ap=[[pstride * part_stride, part_num]] + list(free_ap))


def build_program(debug_taps=False):
    nc = bass.Bass("TRN2", target_bir_lowering=False, debug=False, num_devices=8)

    # ---- DRAM I/O (per core) ----
    grid16 = nc.dram_tensor("grid16", [S, 2, G, G], FP16, kind="ExternalInput")
    w1 = nc.dram_tensor("w1", [18, HID], FP16, kind="ExternalInput")
    b1 = nc.dram_tensor("b1", [HID, 1], FP32, kind="ExternalInput")
    w2a = nc.dram_tensor("w2a", [3, 3, 128, HID], FP16, kind="ExternalInput")
    w2b = nc.dram_tensor("w2b", [3, 66, HID], FP16, kind="ExternalInput")
    b2 = nc.dram_tensor("b2", [HID, 1], FP32, kind="ExternalInput")
    wrta = nc.dram_tensor("wrta", [3, 3, 128, 97], FP16, kind="ExternalInput")
    wrtb = nc.dram_tensor("wrtb", [3, 66, 97], FP16, kind="ExternalInput")
    wa1 = nc.dram_tensor("wa1", [A, HID], FP16, kind="ExternalInput")
    ba1 = nc.dram_tensor("ba1", [HID, 1], FP32, kind="ExternalInput")
    wa2 = nc.dram_tensor("wa2", [HID, A], FP16, kind="ExternalInput")
    ba2 = nc.dram_tensor("ba2", [A, 1], FP32, kind="ExternalInput")
    out = nc.dram_tensor("o", [S, A, G, G], FP16, kind="ExternalOutput")
    if debug_taps:
        dbg_h1 = nc.dram_tensor("dbg_h1", [128, CH, CW], FP16, kind="ExternalOutput")
        dbg_h = nc.dram_tensor("dbg_h", [128, CH, CW], FP16, kind="ExternalOutput")
        dbg_e = nc.dram_tensor("dbg_e", [72, PH, PW], FP16, kind="ExternalOutput")
        dbg_sT = nc.dram_tensor("dbg_sT", [128, 9, PH, PW], FP16, kind="ExternalOutput")
        dbg_r32 = nc.dram_tensor("dbg_r32", [16, PH, PW], FP16, kind="ExternalOutput")
        dbg_V = nc.dram_tensor("dbg_V", [128, CH, CW], FP16, kind="ExternalOutput")
        dbg_q = nc.dram_tensor("dbg_q", [128, PH, PW], FP16, kind="ExternalOutput")

    NCH1 = [(0, 10), (10, 10), (20, 10), (30, 10), (40, 9)]           # 49 rows
    NCH2 = [(0, 9), (9, 9), (18, 9), (27, 9), (36, 9), (45, 6)]       # 51 rows
    MLPN = [(0, 10), (10, 10), (20, 10), (30, 10), (40, 9)]  # row chunks of 49

    with tile.TileContext(nc) as tc:
        import contextlib
        with contextlib.ExitStack() as ctx:
            persist = ctx.enter_context(tc.tile_pool(name="persist", bufs=1))


            # ---------- persistent tiles (scan block layout) ----------
            # partition p = s*8 + rb owns global rows 7rb..7rb+6 of sample s.
            # sTk is a-major [a, k, r, c] (matching the conv's (a,k) output
            # channel order, so the scatter DMA balances); one DVE op per
            # di-group covers all 8 actions with a stride-0 broadcast of the
            # V window.
            sTk = persist.tile([128, A, 9, RC], FP16, tag="sTk")
            Vt = persist.tile([128, RPB + 2, VC], FP16, tag="Vt")
            Rt = persist.tile([128, RPB, PW], FP16, tag="Rt")
            qL = persist.tile([128, A, RC], FP16, tag="qL")

            # grid canvas first: the X1 staging for sample 0 depends on it,
            # so it must not queue behind ~15 weight loads
            Gc = persist.tile([32, CH, CW], FP16, tag="Gc")
            nc.vector.memset(Gc.rearrange("p a b -> p (a b)"), 0.0)
            for ci in range(2):
                eng = nc.sync if ci == 0 else nc.scalar
                eng.dma_start(out=Gc[16 * ci:16 * ci + 16, 1:1 + G, 1:1 + G],
                              in_=grid16[:, ci, :, :])

            # weights in SBUF
            w1t = persist.tile([18, HID], FP16, tag="w1t")
            nc.sync.dma_start(out=w1t, in_=w1[:, :])
            b1A = persist.tile([128, 1], FP32, tag="b1A")
            b1B = persist.tile([22, 1], FP32, tag="b1B")
            nc.sync.dma_start(out=b1A, in_=b1[0:128, :])
            nc.sync.dma_start(out=b1B, in_=b1[128:150, :])
            b2A = persist.tile([128, 1], FP32, tag="b2A")
            b2B = persist.tile([22, 1], FP32, tag="b2B")
            nc.sync.dma_start(out=b2A, in_=b2[0:128, :])
            nc.sync.dma_start(out=b2B, in_=b2[128:150, :])
            # conv2/rt weights: 9 full-K A tiles (ci 0..127, dj applied via rhs
            # column offset at matmul time) + 3 packed B tiles (3dj x 22 ci)
            w2t = []   # [di][dj] -> [128, 150]
            wrtt = []  # [di][dj] -> [128, 97]
            w2tB = []  # [di] -> [66, 150]
            wrttB = []
            for di in range(3):
                w2t.append([])
                wrtt.append([])
                for dj in range(3):
                    t2 = persist.tile([128, HID], FP16, tag=f"w2_{di}_{dj}", name=f"w2_{di}_{dj}")
                    nc.sync.dma_start(out=t2, in_=w2a[di, dj, :, :])
                    w2t[di].append(t2)
                    t3 = persist.tile([128, 97], FP16, tag=f"wrt_{di}_{dj}", name=f"wrt_{di}_{dj}")
                    nc.sync.dma_start(out=t3, in_=wrta[di, dj, :, :])
                    wrtt[di].append(t3)
                tb = persist.tile([66, HID], FP16, tag=f"w2B_{di}", name=f"w2B_{di}")
                nc.sync.dma_start(out=tb, in_=w2b[di, :, :])
                w2tB.append(tb)
                tb2 = persist.tile([66, 97], FP16, tag=f"wrtB_{di}", name=f"wrtB_{di}")
                nc.sync.dma_start(out=tb2, in_=wrtb[di, :, :])
                wrttB.append(tb2)
            wa1t = persist.tile([A, HID], FP16, tag="wa1t")
            nc.sync.dma_start(out=wa1t, in_=wa1[:, :])
            wa2A = persist.tile([128, A], FP16, tag="wa2A")
            wa2B = persist.tile([22, A], FP16, tag="wa2B")
            nc.sync.dma_start(out=wa2A, in_=wa2[0:128, :])
            nc.sync.dma_start(out=wa2B, in_=wa2[128:150, :])
            ba1A = persist.tile([128, 1], FP32, tag="ba1A")
            ba1B = persist.tile([22, 1], FP32, tag="ba1B")
            nc.sync.dma_start(out=ba1A, in_=ba1[0:128, :])
            nc.sync.dma_start(out=ba1B, in_=ba1[128:150, :])
            ba2t = persist.tile([A, 1], FP32, tag="ba2t")
            nc.sync.dma_start(out=ba2t, in_=ba2[:, :])

            # ---------- conv phase (scoped pool, freed before scan) ----------
            with tc.tile_pool(name="convfix", bufs=1) as cfix, \
                 tc.tile_pool(name="cpsum", bufs=4, space="PSUM") as psum:
                # double-buffered X1 (built one iteration ahead so conv1 never
                # waits on the DMA queues); double-buffered h1/h
                X1 = [cfix.tile([18, CH, CW], FP16, tag=f"X1_{i}", name=f"X1_{i}")
                      for i in range(2)]
                h1A = [cfix.tile([128, CH, CW], FP16, tag=f"h1A{i}", name=f"h1A{i}") for i in range(2)]
                h1B = [cfix.tile([22, CH, CW], FP16, tag=f"h1B{i}", name=f"h1B{i}") for i in range(2)]
                hA = [cfix.tile([128, CH, CW], FP16, tag=f"hA{i}", name=f"hA{i}") for i in range(2)]
                hB = [cfix.tile([22, CH, CW], FP16, tag=f"hB{i}", name=f"hB{i}") for i in range(2)]
                # small pre-shifted canvases for the 22-channel B halves only
                # (3 dj-shifts x 22 ci rows packed into 66 partitions); the
                # 128-channel A halves read h1A/hA directly with the dj shift
                # applied via the matmul rhs column offset.
                XB2 = cfix.tile([66, CH, CW], FP16, tag="XB2", name="XB2")
                XBh = cfix.tile([66, CH, CW], FP16, tag="XBh", name="XBh")
                e_s = cfix.tile([72, PH, PW], FP16, tag="e0", name="e0")
                r_s = cfix.tile([1, PH, PW], FP16, tag="r0", name="r0")
                for i in range(2):
                    nc.vector.memset(X1[i].rearrange("p a b -> p (a b)"), 0.0)
                    for t in (h1A[i], h1B[i], hA[i], hB[i]):
                        nc.vector.memset(t.rearrange("p a b -> p (a b)"), 0.0)
                nc.vector.memset(XB2.rearrange("p a b -> p (a b)"), 0.0)
                nc.vector.memset(XBh.rearrange("p a b -> p (a b)"), 0.0)
                nc.vector.memset(e_s.rearrange("p a b -> p (a b)"), 0.0)
                nc.vector.memset(r_s.rearrange("p a b -> p (a b)"), 0.0)

                qidx = [0]

                def build_xb(dst, src_b):
                    for dj in range(3):
                        soff = dj - 1
                        d0c = max(0, -soff)
                        ln = CFLAT - abs(soff)
                        dstp = _part_ap(dst, dj * 22, 1, 22, [[1, ln]],
                                        extra_off=d0c)
                        srcp = _part_ap(src_b, 0, 1, 22, [[1, ln]],
                                        extra_off=max(0, soff))
                        eng = nc.scalar if qidx[0] % 2 == 0 else nc.sync
                        qidx[0] += 1
                        eng.dma_start(out=dstp, in_=srcp)

                def emit_x1(s):
                    xb = X1[s % 2]
                    for di in range(3):
                        for dj in range(3):
                            off = di * CW + dj
                            ln = CFLAT - off
                            dst = _part_ap(xb, di * 3 + dj, 9, 2, [[1, ln]])
                            srcp = _part_ap(Gc, s, 16, 2, [[1, ln]], extra_off=off)
                            eng = nc.scalar if qidx[0] % 2 == 0 else nc.sync
                            qidx[0] += 1
                            eng.dma_start(out=dst, in_=srcp)

                def emit_conv1(s):
                    pp = s % 2
                    xb = X1[s % 2]
                    for (y0, ny) in NCH1:
                        ps = psum.tile([128, 10, G], FP32, tag="psA", name=f"c1ps{s}_{y0}")
                        nc.tensor.matmul(out=ps[:, 0:ny, :],
                                         lhsT=w1t[:, 0:128],
                                         rhs=xb[:, y0:y0 + ny, 0:G],
                                         start=True, stop=True)
                        nc.scalar.activation(out=h1A[pp][:, 1 + y0:1 + y0 + ny, 2:2 + G],
                                             in_=ps[:, 0:ny, 0:G], func=AF.Relu,
                                             bias=b1A, scale=1.0)
                        ps2 = psum.tile([22, 10, G], FP32, tag="psB", name=f"c1ps2{s}_{y0}")
                        nc.tensor.matmul(out=ps2[:, 0:ny, :],
                                         lhsT=w1t[:, 128:150],
                                         rhs=xb[:, y0:y0 + ny, 0:G],
                                         start=True, stop=True)
                        nc.scalar.activation(out=h1B[pp][:, 1 + y0:1 + y0 + ny, 2:2 + G],
                                             in_=ps2[:, 0:ny, 0:G], func=AF.Relu,
                                             bias=b1B, scale=1.0)

                def emit_conv2(s):
                    # K-tile-outer over 3-chunk groups: one LDWEIGHTS per
                    # (K-tile, half, group) serves 3 row-chunks, so the weight
                    # load hides fully under the 3 preceding matmuls (the
                    # baseline paid ~90ns of exposed LDWEIGHTS every matmul
                    # group). 3 live PSUM banks per tag keeps within the
                    # 8-bank budget.
                    pp = s % 2
                    for grp in (NCH1[0:3], NCH1[3:5]):
                        for (lo, hi, ht, bias, tag) in (
                                (0, 128, hA, b2A, "psA"),
                                (128, 150, hB, b2B, "psB")):
                            nco = hi - lo
                            pss = [psum.tile([nco, 10, G], FP32, tag=tag,
                                             name=f"c2{tag}{s}_{y0}")
                                   for (y0, ny) in grp]
                            for di in range(3):
                                for dj in range(3):
                                    first = (di == 0 and dj == 0)
                                    for ci, (y0, ny) in enumerate(grp):
                                        rhs = h1A[pp][:, y0 + di:y0 + di + ny,
                                                      1 + dj:1 + dj + G]
                                        nc.tensor.matmul(out=pss[ci][:, 0:ny, :],
                                                         lhsT=w2t[di][dj][:, lo:hi],
                                                         rhs=rhs,
                                                         start=first, stop=False)
                            for di in range(3):
                                last = (di == 2)
                                for ci, (y0, ny) in enumerate(grp):
                                    rhs = XB2[:, y0 + di:y0 + di + ny, 2:2 + G]
                                    nc.tensor.matmul(out=pss[ci][:, 0:ny, :],
                                                     lhsT=w2tB[di][:, lo:hi], rhs=rhs,
                                                     start=False, stop=last)
                            for ci, (y0, ny) in enumerate(grp):
                                nc.scalar.activation(out=ht[pp][:, 2 + y0:2 + y0 + ny, 2:2 + G],
                                                     in_=pss[ci][:, 0:ny, 0:G], func=AF.Relu,
                                                     bias=bias, scale=1.0)

                def emit_rt(s):
                    pp = s % 2
                    for grp in (NCH2[0:3], NCH2[3:6]):
                        pss = [psum.tile([97, 9, 51], FP32, tag="psA",
                                         name=f"rtps{s}_{y0}")
                               for (y0, ny) in grp]
                        for di in range(3):
                            for dj in range(3):
                                first = (di == 0 and dj == 0)
                                for ci, (y0, ny) in enumerate(grp):
                                    rhs = hA[pp][:, y0 + di:y0 + di + ny, dj:dj + 51]
                                    nc.tensor.matmul(out=pss[ci][:, 0:ny, :],
                                                     lhsT=wrtt[di][dj], rhs=rhs,
                                                     start=first, stop=False)
                        for di in range(3):
                            last = (di == 2)
                            for ci, (y0, ny) in enumerate(grp):
                                rhs = XBh[:, y0 + di:y0 + di + ny, 1:1 + 51]
                                nc.tensor.matmul(out=pss[ci][:, 0:ny, :],
                                                 lhsT=wrttB[di], rhs=rhs,
                                                 start=False, stop=last)
                        for ci, (y0, ny) in enumerate(grp):
                            nc.scalar.activation(out=e_s[:, y0:y0 + ny, 0:51],
                                                 in_=pss[ci][0:72, 0:ny, :], func=AF.Exp,
                                                 bias=0.0, scale=1.0)
                            nc.scalar.activation(out=r_s[:, y0:y0 + ny, 0:51],
                                                 in_=pss[ci][96:97, 0:ny, :], func=AF.Copy,
                                                 bias=0.0, scale=1.0)
                    # scatter e into sTk block layout: one DMA per row-block
                    # (72 src partitions -> one dst partition, (a,k) order
                    # transposed to (k,a) via two dst free dims).
                    sTf = sTk[:, :, :, :]
                    spstr = sTf.ap[0][0]        # A*9*RC
                    esf = e_s[:, :, :]
                    for rb in range(RB):
                        ln = RC if rb < 7 else 2 * PW   # rb 7 has rows 49,50
                        dst = bass.AP(tensor=sTf.tensor,
                                      offset=sTf.offset + (s * RB + rb) * spstr,
                                      ap=[[spstr, 1], [RC, 72], [1, ln]])
                        src = bass.AP(tensor=esf.tensor,
                                      offset=esf.offset + rb * RC,
                                      ap=[[PFLAT, 72], [1, ln]])
                        eng = nc.scalar if rb % 2 == 0 else nc.sync
                        eng.dma_start(out=dst, in_=src)
                    # reward + V seed (V0=0 makes step 1 trivially V1=R, so
                    # seed V's own rows with R and run one fewer step).
                    Rtf = Rt[:, :, :]
                    rpstr = Rtf.ap[0][0]        # RPB*PW
                    rsf = r_s[:, :, :]
                    Vtf = Vt[:, :, :]
                    vpstr = Vtf.ap[0][0]        # (RPB+2)*VC
                    nc.sync.dma_start(
                        out=bass.AP(tensor=Rtf.tensor,
                                    offset=Rtf.offset + (s * RB) * rpstr,
                                    ap=[[rpstr, 7], [1, RC]]),
                        in_=bass.AP(tensor=rsf.tensor, offset=rsf.offset,
                                    ap=[[PFLAT, 1], [RC, 7], [1, RC]]))
                    nc.sync.dma_start(
                        out=bass.AP(tensor=Rtf.tensor,
                                    offset=Rtf.offset + (s * RB + 7) * rpstr,
                                    ap=[[rpstr, 1], [1, 2 * PW]]),
                        in_=bass.AP(tensor=rsf.tensor, offset=rsf.offset + 7 * RC,
                                    ap=[[PFLAT, 1], [1, 2 * PW]]))
                    nc.scalar.dma_start(
                        out=bass.AP(tensor=Vtf.tensor,
                                    offset=Vtf.offset + (s * RB) * vpstr + VC + 1,
                                    ap=[[vpstr, 7], [VC, RPB], [1, PW]]),
                        in_=bass.AP(tensor=rsf.tensor, offset=rsf.offset,
                                    ap=[[PFLAT, 1], [RC, 7], [PW, RPB], [1, PW]]))
                    nc.scalar.dma_start(
                        out=bass.AP(tensor=Vtf.tensor,
                                    offset=Vtf.offset + (s * RB + 7) * vpstr + VC + 1,
                                    ap=[[vpstr, 1], [VC, 2], [1, PW]]),
                        in_=bass.AP(tensor=rsf.tensor, offset=rsf.offset + 7 * RC,
                                    ap=[[PFLAT, 1], [PW, 2], [1, PW]]))

                # software pipeline: PE stream is conv2(s), conv1(s+1),
                # rt(s-1). Delaying rt by one sample gives every X3 build a
                # full PE-iteration of slack, so the DMA builds never stall
                # the PE (the baseline lost ~430us to such gaps).
                emit_x1(0)
                emit_x1(1)
                emit_conv1(0)
                build_xb(XB2, h1B[0])
                # scan-tile zero-fill: 33us of DVE memset, emitted after the
                # conv head so it runs under conv1/conv2(0) instead of
                # delaying the first matmul's canvas setup. Only needed
                # before the first rt scatter.
                nc.vector.memset(sTk.rearrange("p a b c -> p (a b c)"), 0.0)
                nc.vector.memset(Vt.rearrange("p a b -> p (a b)"), 0.0)
                nc.vector.memset(Rt.rearrange("p a b -> p (a b)"), 0.0)
                for s in range(S):
                    emit_conv2(s)
                    if s + 1 < S:
                        emit_conv1(s + 1)
                        if s + 2 < S:
                            emit_x1(s + 2)
                        build_xb(XB2, h1B[(s + 1) % 2])
                    if s >= 1:
                        emit_rt(s - 1)
                    build_xb(XBh, hB[s % 2])
                    if debug_taps and s == 0:
                        nc.sync.dma_start(out=dbg_h1[:, :, :], in_=h1A[0][:, :, :])
                        nc.sync.dma_start(out=dbg_h[:, :, :], in_=hA[0][:, :, :])
                        nc.sync.dma_start(out=dbg_e[:, :, :], in_=e_s[:, :, :])
                emit_rt(S - 1)

            # ---------- softmax over k (block layout, k-major) ----------
            # denominators via a k-tree, +eps so the zero pad rows give a
            # finite reciprocal (0 * big = 0, not 0 * inf = NaN), then one
            # broadcast multiply normalizes all 9 k-planes.
            with tc.tile_pool(name="smx", bufs=1) as smx:
                t1 = smx.tile([128, A, 4, RC], FP16, tag="sx1")
                nc.vector.tensor_add(out=t1[:, :, :, :], in0=sTk[:, :, 0:4, :],
                                     in1=sTk[:, :, 4:8, :])
                nc.vector.tensor_add(out=t1[:, :, 0:2, :], in0=t1[:, :, 0:2, :],
                                     in1=t1[:, :, 2:4, :])
                nc.vector.tensor_add(out=t1[:, :, 0, :], in0=t1[:, :, 0, :],
                                     in1=t1[:, :, 1, :])
                nc.vector.tensor_add(out=t1[:, :, 0, :], in0=t1[:, :, 0, :],
                                     in1=sTk[:, :, 8, :])
                rec = smx.tile([128, A, RC], FP16, tag="sxr")
                lnD = smx.tile([128, A, RC], FP16, tag="sxln")
                epsb = smx.tile([128, 1], FP32, tag="sxeps")
                nc.vector.memset(epsb[:, :], 1e-3)
                nc.scalar.activation(out=lnD[:, :, :], in_=t1[:, :, 0, :],
                                     func=AF.Ln, bias=epsb[:, :], scale=1.0)
                nc.scalar.activation(out=rec[:, :, :], in_=lnD[:, :, :],
                                     func=AF.Exp, bias=0.0, scale=-1.0)
                sTf = sTk[:, :, :, :]
                spstr = sTf.ap[0][0]
                recf = rec[:, :, :]
                nc.vector.tensor_mul(
                    out=bass.AP(tensor=sTf.tensor, offset=sTf.offset,
                                ap=[[spstr, 128], [9 * RC, A], [RC, 9], [1, RC]]),
                    in0=bass.AP(tensor=sTf.tensor, offset=sTf.offset,
                                ap=[[spstr, 128], [9 * RC, A], [RC, 9], [1, RC]]),
                    in1=bass.AP(tensor=recf.tensor, offset=recf.offset,
                                ap=[[recf.ap[0][0], 128], [RC, A], [0, 9], [1, RC]]))

            # ---------- value-iteration scan (block layout) ----------
            # Per step: 2 halo DMAs (hidden under the di=1 multiply), 3
            # stride-0-broadcast multiplies (one per di), a 4-op k-sum tree,
            # a 3-op in-free-dim action max, and the reward add writing V's
            # own rows. No cross-partition max tree, no V replication.
            Vtf = Vt[:, :, :]
            vpstr = Vtf.ap[0][0]
            sTf = sTk[:, :, :, :]
            spstr = sTf.ap[0][0]

            # the DVE ISA allows at most 3 free dims, so the multiply is one
            # op per k = (di, dj): free dims (a broadcast, row, col)
            def vwin(di, dj):
                return bass.AP(tensor=Vtf.tensor,
                               offset=Vtf.offset + di * VC + dj,
                               ap=[[vpstr, 128], [0, A], [VC, RPB], [1, PW]])

            def kplane(full, k):
                return bass.AP(tensor=full.tensor, offset=full.offset + k * RC,
                               ap=[[full.ap[0][0], 128], [9 * RC, A],
                                   [PW, RPB], [1, PW]])

            with tc.tile_pool(name="scan", bufs=1) as scan, \
                 tc.tile_pool(name="scantail", bufs=2) as tail:
                P = scan.tile([128, A, 9, RC], FP16, tag="P")
                Rtf = Rt[:, :, :]
                rpstr = Rtf.ap[0][0]
                for t in range(1, KST):
                    # halo exchange from the V of the previous step: slot 0
                    # <- previous partition's last own row, slot 8 <- next
                    # partition's first own row. Runs on both HWDGE queues
                    # under the (halo-free) di=1 multiply.
                    nc.sync.dma_start(
                        out=bass.AP(tensor=Vtf.tensor,
                                    offset=Vtf.offset + vpstr,
                                    ap=[[vpstr, 127], [1, VC]]),
                        in_=bass.AP(tensor=Vtf.tensor,
                                    offset=Vtf.offset + 7 * VC,
                                    ap=[[vpstr, 127], [1, VC]]))
                    nc.scalar.dma_start(
                        out=bass.AP(tensor=Vtf.tensor,
                                    offset=Vtf.offset + 8 * VC,
                                    ap=[[vpstr, 127], [1, VC]]),
                        in_=bass.AP(tensor=Vtf.tensor,
                                    offset=Vtf.offset + vpstr + VC,
                                    ap=[[vpstr, 127], [1, VC]]))
                    Pf = P[:, :, :, :]
                    for di in (1, 0, 2):
                        for dj in range(3):
                            k = di * 3 + dj
                            nc.vector.tensor_mul(out=kplane(Pf, k),
                                                 in0=kplane(sTf, k),
                                                 in1=vwin(di, dj))
                    # k-sum tree (4 ops)
                    nc.vector.tensor_add(out=P[:, :, 0:4, :], in0=P[:, :, 0:4, :],
                                         in1=P[:, :, 4:8, :])
                    nc.vector.tensor_add(out=P[:, :, 0:2, :], in0=P[:, :, 0:2, :],
                                         in1=P[:, :, 2:4, :])
                    nc.vector.tensor_add(out=P[:, :, 0, :], in0=P[:, :, 0, :],
                                         in1=P[:, :, 1, :])
                    nc.vector.tensor_add(out=P[:, :, 0, :], in0=P[:, :, 0, :],
                                         in1=P[:, :, 8, :])
                    if t == KST - 1:
                        nc.vector.tensor_add(
                            out=qL[:, :, :], in0=P[:, :, 0, :],
                            in1=bass.AP(tensor=Rtf.tensor, offset=Rtf.offset,
                                        ap=[[rpstr, 128], [0, A], [1, RC]]))
                        break
                    # action max in the free dim (in place on P[:, :, 0])
                    nc.vector.tensor_max(out=P[:, 0:4, 0, :], in0=P[:, 0:4, 0, :],
                                         in1=P[:, 4:8, 0, :])
                    nc.vector.tensor_max(out=P[:, 0:2, 0, :], in0=P[:, 0:2, 0, :],
                                         in1=P[:, 2:4, 0, :])
                    vm = tail.tile([128, RPB, PW], FP16, tag="vm")
                    nc.vector.tensor_max(out=vm.rearrange("p a b -> p (a b)"),
                                         in0=P[:, 0, 0, :], in1=P[:, 1, 0, :])
                    # reward add writes V's own rows (pad rows stay 0: their
                    # sT and R are 0, so max over a gives 0 + 0)
                    nc.vector.tensor_add(out=Vt[:, 1:1 + RPB, 1:1 + PW],
                                         in0=vm[:, :, :], in1=Rt[:, :, :])

            # ---------- per-pixel MLP ----------
            with tc.tile_pool(name="mlp", bufs=4) as mlp, \
                 tc.tile_pool(name="mpsum", bufs=3, space="PSUM") as mpsum:
                qLf = qL[:, :, :]
                qpstr = qLf.ap[0][0]
                # gather all samples' q into one [a, s, 56-row] staging tile
                # with 32 wide DMAs (8 actions x 4 sample groups), all on the
                # sync queue: the scalar queue stays free for the MLP's
                # activations (a dma_start costs ~730ns of queue-engine time,
                # so per-sample-per-action gathers starve the Act engine).
                qAll = mlp.tile([A, S, RB * RPB, PW], FP16, tag="qAll", bufs=1)
                qAf = qAll[:, :, :, :]
                qapstr = qAf.ap[0][0]
                for g in range(4):
                    for a in range(A):
                        dst = bass.AP(tensor=qAf.tensor,
                                      offset=qAf.offset + a * qapstr
                                      + g * 4 * RB * RC,
                                      ap=[[qapstr, 1], [RC, 32], [1, RC]])
                        srcp = bass.AP(tensor=qLf.tensor,
                                      offset=qLf.offset + g * 32 * qpstr + a * RC,
                                      ap=[[qpstr, 32], [1, RC]])
                        eng = (nc.scalar if (g == 0 and a % 2 == 1)
                               else nc.sync)
                        eng.dma_start(out=dst, in_=srcp)
                for s in range(S):
                    midA = mlp.tile([128, G, G], FP16, tag="midA", bufs=2)
                    midB = mlp.tile([22, G, G], FP16, tag="midB", bufs=2)
                    # half-outer order: one LDWEIGHTS per co-half instead of
                    # alternating weights every matmul
                    for (y0, ny) in MLPN:
                        rhs = qAll[:, s, y0:y0 + ny, 0:G]
                        p1 = mpsum.tile([128, 10, G], FP32, tag="m1")
                        nc.tensor.matmul(out=p1[:, 0:ny, :],
                                         lhsT=wa1t[:, 0:128], rhs=rhs,
                                         start=True, stop=True)
                        nc.scalar.activation(out=midA[:, y0:y0 + ny, :],
                                             in_=p1[:, 0:ny, :], func=AF.Relu,
                                             bias=ba1A, scale=1.0)
                    for (y0, ny) in MLPN:
                        rhs = qAll[:, s, y0:y0 + ny, 0:G]
                        p2 = mpsum.tile([22, 10, G], FP32, tag="m2", bufs=2)
                        nc.tensor.matmul(out=p2[:, 0:ny, :],
                                         lhsT=wa1t[:, 128:150], rhs=rhs,
                                         start=True, stop=True)
                        nc.vector.tensor_scalar(
                            out=midB[:, y0:y0 + ny, :],
                            in0=p2[:, 0:ny, :], scalar1=ba1B, scalar2=0.0,
                            op0=mybir.AluOpType.add, op1=mybir.AluOpType.max)
                    ost = mlp.tile([A, G, G], FP16, tag="ost", bufs=2)
                    for ci, (y0, ny) in enumerate(MLPN):
                        p3 = mpsum.tile([A, 10, G], FP32, tag="m3")
                        nc.tensor.matmul(out=p3[:, 0:ny, :], lhsT=wa2A,
                                         rhs=midA[:, y0:y0 + ny, :],
                                         start=True, stop=False)
                        nc.tensor.matmul(out=p3[:, 0:ny, :], lhsT=wa2B,
                                         rhs=midB[:, y0:y0 + ny, :],
                                         start=False, stop=True)
                        # balance the PSUM drains: Act ends up with 8
                        # chunks/sample (5 midA + 3 ost) at ~600ns, DVE with 7
                        # (5 midB + 2 ost) at ~680ns
                        if ci % 2 == 0:
                            nc.scalar.activation(out=ost[:, y0:y0 + ny, :],
                                                 in_=p3[:, 0:ny, :],
                                                 func=AF.Identity,
                                                 bias=ba2t, scale=1.0)
                        else:
                            nc.vector.tensor_scalar(
                                out=ost[:, y0:y0 + ny, :],
                                in0=p3[:, 0:ny, :], scalar1=ba2t, scalar2=None,
                                op0=mybir.AluOpType.add)
                    if s == S - 1:
                        nc.scalar.dma_start(out=out[s, 0:4, :, :],
                                            in_=ost[0:4, :, :])
                        nc.sync.dma_start(out=out[s, 4:8, :, :],
                                          in_=ost[4:8, :, :])
                    else:
                        eng = nc.scalar if s % 2 == 0 else nc.sync
                        eng.dma_start(
                            out=out[s, :, :, :],
                            in_=ost[:, :, :])

    _split_multi_waits(nc)
    return nc


def _prep_weights(inputs):
    f32 = lambda x: np.asarray(x, dtype=np.float32)
    h1_w = f32(inputs["h1_w"]); h1_b = f32(inputs["h1_b"])
    h2_w = f32(inputs["h2_w"]); h2_b = f32(inputs["h2_b"])
    r_w = f32(inputs["r_w"]); t_w = f32(inputs["t_w"])
    a1_w = f32(inputs["a1_w"]); a1_b = f32(inputs["a1_b"])
    a2_w = f32(inputs["a2_w"]); a2_b = f32(inputs["a2_b"])

    # conv1: k = ci*9 + di*3 + dj
    w1 = h1_w.transpose(1, 2, 3, 0).reshape(18, HID).astype(np.float16)
    # conv2 / rt: A tiles [di, dj, ci<128, co]; B tiles [di, (dj, ci>=128), co]
    w2f = np.ascontiguousarray(
        h2_w.transpose(2, 3, 1, 0), dtype=np.float32)   # [di, dj, ci, co]
    w2A = w2f[:, :, 0:128, :].astype(np.float16)
    w2B = np.ascontiguousarray(
        w2f[:, :, 128:150, :]).reshape(3, 66, HID).astype(np.float16)
    wrt_full = np.zeros((3, 3, HID, 97), np.float32)
    wrt_full[:, :, :, 0:72] = t_w.transpose(2, 3, 1, 0)  # [di, dj, ci, m]
    wrt_full[:, :, :, 96:97] = r_w.transpose(2, 3, 1, 0)
    wrtA = wrt_full[:, :, 0:128, :].astype(np.float16)
    wrtB = np.ascontiguousarray(
        wrt_full[:, :, 128:150, :]).reshape(3, 66, 97).astype(np.float16)
    return {
        "w1": w1, "b1": h1_b.reshape(HID, 1),
        "w2a": w2A, "w2b": w2B, "b2": h2_b.reshape(HID, 1),
        "wrta": wrtA, "wrtb": wrtB,
        "wa1": a1_w.T.astype(np.float16).copy(),      # [8, 150]
        "ba1": a1_b.reshape(HID, 1),
        "wa2": a2_w.T.astype(np.float16).copy(),      # [150, 8]
        "ba2": a2_b.reshape(A, 1),
    }


_CACHE = {}


def _get_program():
    if "nc" not in _CACHE:
        _CACHE["nc"] = build_program()
    return _CACHE["nc"]


def kernel(**inputs):
    nc = _get_program()
    grid = np.asarray(inputs["grid"], dtype=np.float32)
    wts = _prep_weights(inputs)
    in_maps = []
    for c in range(8):
        m = {"grid16": grid[c * S:(c + 1) * S].astype(np.float16)}
        m.update(wts)
        in_maps.append(m)
    res = run_bass_kernel_spmd(nc, in_maps, core_ids=list(range(8)))
    outp = np.concatenate([res.results[c]["o"] for c in range(8)], axis=0)
    return outp.astype(np.float32)


def run_traced(inputs, tmpdir):
    """Like kernel() but with NTFF profiling; returns (output, exec_time_ns)."""
    import ctypes, contextlib
    sys.path.insert(0, "/root/.axon_site/trn_agent_boot")
    import trn_boot
    hook = trn_boot._ntff_profile_via_ctypes("/opt/axon/libaxon_pjrt.so")
    mod = types.ModuleType("antenv.axon_hooks")
    mod.get_axon_ntff_profile_hook = lambda: hook
    sys.modules["antenv.axon_hooks"] = mod

    nc = _get_program()
    grid = np.asarray(inputs["grid"], dtype=np.float32)
    wts = _prep_weights(inputs)
    in_maps = []
    for c in range(8):
        m = {"grid16": grid[c * S:(c + 1) * S].astype(np.float16)}
        m.update(wts)
        in_maps.append(m)
    res = run_bass_kernel_spmd(nc, in_maps, core_ids=list(range(8)),
                               trace=True, tmpdir=tmpdir)
    outp = np.concatenate([res.results[c]["o"] for c in range(8)], axis=0)
    return outp.astype(np.float32), res.exec_time_ns



# revision 66
# speedup vs baseline: 1.0039x; 1.0039x over previous
"""Trainium2 Bass kernel for the DTVIN (dynamic-transition VIN) model.

Self-contained: accepts FULL inputs (batch 128), shards batch over 8
NeuronCores (16 samples/core, pure data parallel), runs one Bass program
per core via run_bass_kernel_spmd, returns full [128, 8, 49, 49] logits.

Per-core program (all fp16 compute, fp32 PSUM accumulation):
  conv1 (K=18 im2col matmul) -> relu -> conv2 (dj-folded im2col, K=450 x
  3 row-shifts) -> relu -> reward/trans conv (pad=2, 51x51, K=450 x 3) ->
  exp on drain -> softmax (tree-sum + reciprocal) -> 30-step value
  iteration on the Vector engine with (action,sample) partition layout ->
  per-pixel MLP (8->150->8) -> logits.
"""
import sys
import types
import numpy as np

for p in ("/opt/trn_rl_repo", "/root/.axon_site/_ro/trn_rl_repo"):
    if p not in sys.path:
        sys.path.append(p)

import concourse.bass as bass
import concourse.tile as tile
from concourse import mybir
from concourse.bass_utils import run_bass_kernel_spmd

FP16 = mybir.dt.float16
FP32 = mybir.dt.float32
AF = mybir.ActivationFunctionType

# geometry
S = 16          # samples per core
HID = 150
A = 8
G = 49          # conv grid
HP = 51         # value-iteration grid
KST = 30        # value-iteration steps
CH, CW = 53, 54          # big canvas (rows, cols) for conv-side tensors
PH, PW = 51, 51          # plane canvas for sT / scan tensors
CFLAT = CH * CW          # 2862
PFLAT = PH * PW          # 2652
# scan block layout: partition p = s*8 + rb; each partition owns 7 rows
RB = 8                   # row blocks per sample
RPB = 7                  # rows per block (8*7 = 56 >= 51; rows 51..55 zero)
VC = 53                  # V cols: 1 + 51 + 1 zero pads
RC = RPB * PW            # 357 elements per (row-block, col) plane


def _split_multi_waits(nc):
    """This walrus build accepts at most ONE sync wait per instruction.
    Move extra waits onto preceding same-engine NoOps (queues run in
    order, so nop-waits followed by the inst == waiting on all)."""
    n = 0
    for f in nc.m.functions:
        for bb in f.blocks:
            new_insts = []
            for ins in bb.instructions:
                si = ins.sync_info
                if si is not None and len(si.on_wait) > 1:
                    waits = list(si.on_wait)
                    for w in waits[:-1]:
                        n += 1
                        nop = mybir.InstNoOp(name=f"waitsplit_{n}", ins=[], outs=[])
                        nop.engine = ins.engine
                        nop.sync_info = mybir.SyncInfo(on_wait=[w], on_update=[])
                        new_insts.append(nop)
                    ins.sync_info = mybir.SyncInfo(on_wait=[waits[-1]],
                                                   on_update=list(si.on_update))
                new_insts.append(ins)
            bb.instructions = new_insts


def _part_ap(t, part_start, part_stride, part_num, free_ap, extra_off=0):
    """AP over tile `t` with a strided partition dim (for DMA only)."""
    full = t[:, :] if len(t.shape) == 2 else t[:, :, :] if len(t.shape) == 3 else t[:, :, :, :]
    pstride = full.ap[0][0]  # elements per partition step
    return bass.AP(tensor=full.tensor,
                   offset=full.offset + part_start * pstride + extra_off,
                   ap=[[pstride * part_stride, part_num]] + list(free_ap))


def build_program(debug_taps=False):
    nc = bass.Bass("TRN2", target_bir_lowering=False, debug=False, num_devices=8)

    # ---- DRAM I/O (per core) ----
    grid16 = nc.dram_tensor("grid16", [S, 2, G, G], FP16, kind="ExternalInput")
    w1 = nc.dram_tensor("w1", [18, HID], FP16, kind="ExternalInput")
    b1 = nc.dram_tensor("b1", [HID, 1], FP32, kind="ExternalInput")
    w2a = nc.dram_tensor("w2a", [3, 3, 128, HID], FP16, kind="ExternalInput")
    w2b = nc.dram_tensor("w2b", [3, 66, HID], FP16, kind="ExternalInput")
    b2 = nc.dram_tensor("b2", [HID, 1], FP32, kind="ExternalInput")
    wrta = nc.dram_tensor("wrta", [3, 3, 128, 97], FP16, kind="ExternalInput")
    wrtb = nc.dram_tensor("wrtb", [3, 66, 97], FP16, kind="ExternalInput")
    wa1 = nc.dram_tensor("wa1", [A, HID], FP16, kind="ExternalInput")
    ba1 = nc.dram_tensor("ba1", [HID, 1], FP32, kind="ExternalInput")
    wa2 = nc.dram_tensor("wa2", [HID, A], FP16, kind="ExternalInput")
    ba2 = nc.dram_tensor("ba2", [A, 1], FP32, kind="ExternalInput")
    out = nc.dram_tensor("o", [S, A, G, G], FP16, kind="ExternalOutput")
    if debug_taps:
        dbg_h1 = nc.dram_tensor("dbg_h1", [128, CH, CW], FP16, kind="ExternalOutput")
        dbg_h = nc.dram_tensor("dbg_h", [128, CH, CW], FP16, kind="ExternalOutput")
        dbg_e = nc.dram_tensor("dbg_e", [72, PH, PW], FP16, kind="ExternalOutput")
        dbg_sT = nc.dram_tensor("dbg_sT", [128, 9, PH, PW], FP16, kind="ExternalOutput")
        dbg_r32 = nc.dram_tensor("dbg_r32", [16, PH, PW], FP16, kind="ExternalOutput")
        dbg_V = nc.dram_tensor("dbg_V", [128, CH, CW], FP16, kind="ExternalOutput")
        dbg_q = nc.dram_tensor("dbg_q", [128, PH, PW], FP16, kind="ExternalOutput")

    NCH1 = [(0, 10), (10, 10), (20, 10), (30, 10), (40, 9)]           # 49 rows
    NCH2 = [(0, 9), (9, 9), (18, 9), (27, 9), (36, 9), (45, 6)]       # 51 rows
    MLPN = [(0, 10), (10, 10), (20, 10), (30, 10), (40, 9)]  # row chunks of 49

    with tile.TileContext(nc) as tc:
        import contextlib
        with contextlib.ExitStack() as ctx:
            persist = ctx.enter_context(tc.tile_pool(name="persist", bufs=1))


            # ---------- persistent tiles (scan block layout) ----------
            # partition p = s*8 + rb owns global rows 7rb..7rb+6 of sample s.
            # sTk is a-major [a, k, r, c] (matching the conv's (a,k) output
            # channel order, so the scatter DMA balances); one DVE op per
            # di-group covers all 8 actions with a stride-0 broadcast of the
            # V window.
            sTk = persist.tile([128, A, 9, RC], FP16, tag="sTk")
            Vt = persist.tile([128, RPB + 2, VC], FP16, tag="Vt")
            Rt = persist.tile([128, RPB, PW], FP16, tag="Rt")
            qL = persist.tile([128, A, RC], FP16, tag="qL")

            # grid canvas first: the X1 staging for sample 0 depends on it,
            # so it must not queue behind ~15 weight loads
            Gc = persist.tile([32, CH, CW], FP16, tag="Gc")
            nc.vector.memset(Gc.rearrange("p a b -> p (a b)"), 0.0)
            for ci in range(2):
                eng = nc.sync if ci == 0 else nc.scalar
                eng.dma_start(out=Gc[16 * ci:16 * ci + 16, 1:1 + G, 1:1 + G],
                              in_=grid16[:, ci, :, :])

            # weights in SBUF
            w1t = persist.tile([18, HID], FP16, tag="w1t")
            nc.sync.dma_start(out=w1t, in_=w1[:, :])
            b1A = persist.tile([128, 1], FP32, tag="b1A")
            b1B = persist.tile([22, 1], FP32, tag="b1B")
            nc.sync.dma_start(out=b1A, in_=b1[0:128, :])
            nc.sync.dma_start(out=b1B, in_=b1[128:150, :])
            b2A = persist.tile([128, 1], FP32, tag="b2A")
            b2B = persist.tile([22, 1], FP32, tag="b2B")
            nc.sync.dma_start(out=b2A, in_=b2[0:128, :])
            nc.sync.dma_start(out=b2B, in_=b2[128:150, :])
            # conv2/rt weights: 9 full-K A tiles (ci 0..127, dj applied via rhs
            # column offset at matmul time) + 3 packed B tiles (3dj x 22 ci)
            w2t = []   # [di][dj] -> [128, 150]
            wrtt = []  # [di][dj] -> [128, 97]
            w2tB = []  # [di] -> [66, 150]
            wrttB = []
            for di in range(3):
                w2t.append([])
                wrtt.append([])
                for dj in range(3):
                    t2 = persist.tile([128, HID], FP16, tag=f"w2_{di}_{dj}", name=f"w2_{di}_{dj}")
                    nc.sync.dma_start(out=t2, in_=w2a[di, dj, :, :])
                    w2t[di].append(t2)
                    t3 = persist.tile([128, 97], FP16, tag=f"wrt_{di}_{dj}", name=f"wrt_{di}_{dj}")
                    nc.sync.dma_start(out=t3, in_=wrta[di, dj, :, :])
                    wrtt[di].append(t3)
                tb = persist.tile([66, HID], FP16, tag=f"w2B_{di}", name=f"w2B_{di}")
                nc.sync.dma_start(out=tb, in_=w2b[di, :, :])
                w2tB.append(tb)
                tb2 = persist.tile([66, 97], FP16, tag=f"wrtB_{di}", name=f"wrtB_{di}")
                nc.sync.dma_start(out=tb2, in_=wrtb[di, :, :])
                wrttB.append(tb2)
            wa1t = persist.tile([A, HID], FP16, tag="wa1t")
            nc.sync.dma_start(out=wa1t, in_=wa1[:, :])
            wa2A = persist.tile([128, A], FP16, tag="wa2A")
            wa2B = persist.tile([22, A], FP16, tag="wa2B")
            nc.sync.dma_start(out=wa2A, in_=wa2[0:128, :])
            nc.sync.dma_start(out=wa2B, in_=wa2[128:150, :])
            ba1A = persist.tile([128, 1], FP32, tag="ba1A")
            ba1B = persist.tile([22, 1], FP32, tag="ba1B")
            nc.sync.dma_start(out=ba1A, in_=ba1[0:128, :])
            nc.sync.dma_start(out=ba1B, in_=ba1[128:150, :])
            ba2t = persist.tile([A, 1], FP32, tag="ba2t")
            nc.sync.dma_start(out=ba2t, in_=ba2[:, :])

            # ---------- conv phase (scoped pool, freed before scan) ----------
            with tc.tile_pool(name="convfix", bufs=1) as cfix, \
                 tc.tile_pool(name="cpsum", bufs=4, space="PSUM") as psum:
                # double-buffered X1 (built one iteration ahead so conv1 never
                # waits on the DMA queues); double-buffered h1/h
                X1 = [cfix.tile([18, CH, CW], FP16, tag=f"X1_{i}", name=f"X1_{i}")
                      for i in range(2)]
                h1A = [cfix.tile([128, CH, CW], FP16, tag=f"h1A{i}", name=f"h1A{i}") for i in range(2)]
                h1B = [cfix.tile([22, CH, CW], FP16, tag=f"h1B{i}", name=f"h1B{i}") for i in range(2)]
                hA = [cfix.tile([128, CH, CW], FP16, tag=f"hA{i}", name=f"hA{i}") for i in range(2)]
                hB = [cfix.tile([22, CH, CW], FP16, tag=f"hB{i}", name=f"hB{i}") for i in range(2)]
                # small pre-shifted canvases for the 22-channel B halves only
                # (3 dj-shifts x 22 ci rows packed into 66 partitions); the
                # 128-channel A halves read h1A/hA directly with the dj shift
                # applied via the matmul rhs column offset.
                XB2 = cfix.tile([66, CH, CW], FP16, tag="XB2", name="XB2")
                XBh = cfix.tile([66, CH, CW], FP16, tag="XBh", name="XBh")
                e_s = cfix.tile([72, PH, PW], FP16, tag="e0", name="e0")
                r_s = cfix.tile([1, PH, PW], FP16, tag="r0", name="r0")
                for i in range(2):
                    nc.vector.memset(X1[i].rearrange("p a b -> p (a b)"), 0.0)
                    for t in (h1A[i], h1B[i], hA[i], hB[i]):
                        nc.vector.memset(t.rearrange("p a b -> p (a b)"), 0.0)
                nc.vector.memset(XB2.rearrange("p a b -> p (a b)"), 0.0)
                nc.vector.memset(XBh.rearrange("p a b -> p (a b)"), 0.0)
                nc.vector.memset(e_s.rearrange("p a b -> p (a b)"), 0.0)
                nc.vector.memset(r_s.rearrange("p a b -> p (a b)"), 0.0)

                qidx = [0]

                def build_xb(dst, src_b):
                    for dj in range(3):
                        soff = dj - 1
                        d0c = max(0, -soff)
                        ln = CFLAT - abs(soff)
                        dstp = _part_ap(dst, dj * 22, 1, 22, [[1, ln]],
                                        extra_off=d0c)
                        srcp = _part_ap(src_b, 0, 1, 22, [[1, ln]],
                                        extra_off=max(0, soff))
                        eng = nc.scalar if qidx[0] % 2 == 0 else nc.sync
                        qidx[0] += 1
                        eng.dma_start(out=dstp, in_=srcp)

                def emit_x1(s):
                    xb = X1[s % 2]
                    for di in range(3):
                        for dj in range(3):
                            off = di * CW + dj
                            ln = CFLAT - off
                            dst = _part_ap(xb, di * 3 + dj, 9, 2, [[1, ln]])
                            srcp = _part_ap(Gc, s, 16, 2, [[1, ln]], extra_off=off)
                            eng = nc.scalar if qidx[0] % 2 == 0 else nc.sync
                            qidx[0] += 1
                            eng.dma_start(out=dst, in_=srcp)

                def emit_conv1(s):
                    pp = s % 2
                    xb = X1[s % 2]
                    for (y0, ny) in NCH1:
                        ps = psum.tile([128, 10, G], FP32, tag="psA", name=f"c1ps{s}_{y0}")
                        nc.tensor.matmul(out=ps[:, 0:ny, :],
                                         lhsT=w1t[:, 0:128],
                                         rhs=xb[:, y0:y0 + ny, 0:G],
                                         start=True, stop=True)
                        nc.scalar.activation(out=h1A[pp][:, 1 + y0:1 + y0 + ny, 2:2 + G],
                                             in_=ps[:, 0:ny, 0:G], func=AF.Relu,
                                             bias=b1A, scale=1.0)
                        ps2 = psum.tile([22, 10, G], FP32, tag="psB", name=f"c1ps2{s}_{y0}")
                        nc.tensor.matmul(out=ps2[:, 0:ny, :],
                                         lhsT=w1t[:, 128:150],
                                         rhs=xb[:, y0:y0 + ny, 0:G],
                                         start=True, stop=True)
                        nc.scalar.activation(out=h1B[pp][:, 1 + y0:1 + y0 + ny, 2:2 + G],
                                             in_=ps2[:, 0:ny, 0:G], func=AF.Relu,
                                             bias=b1B, scale=1.0)

                def emit_conv2(s):
                    # K-tile-outer over 3-chunk groups: one LDWEIGHTS per
                    # (K-tile, half, group) serves 3 row-chunks, so the weight
                    # load hides fully under the 3 preceding matmuls (the
                    # baseline paid ~90ns of exposed LDWEIGHTS every matmul
                    # group). 3 live PSUM banks per tag keeps within the
                    # 8-bank budget.
                    pp = s % 2
                    for grp in (NCH1[0:3], NCH1[3:5]):
                        for (lo, hi, ht, bias, tag) in (
                                (0, 128, hA, b2A, "psA"),
                                (128, 150, hB, b2B, "psB")):
                            nco = hi - lo
                            pss = [psum.tile([nco, 10, G], FP32, tag=tag,
                                             name=f"c2{tag}{s}_{y0}")
                                   for (y0, ny) in grp]
                            for di in range(3):
                                for dj in range(3):
                                    first = (di == 0 and dj == 0)
                                    for ci, (y0, ny) in enumerate(grp):
                                        rhs = h1A[pp][:, y0 + di:y0 + di + ny,
                                                      1 + dj:1 + dj + G]
                                        nc.tensor.matmul(out=pss[ci][:, 0:ny, :],
                                                         lhsT=w2t[di][dj][:, lo:hi],
                                                         rhs=rhs,
                                                         start=first, stop=False)
                            for di in range(3):
                                last = (di == 2)
                                for ci, (y0, ny) in enumerate(grp):
                                    rhs = XB2[:, y0 + di:y0 + di + ny, 2:2 + G]
                                    nc.tensor.matmul(out=pss[ci][:, 0:ny, :],
                                                     lhsT=w2tB[di][:, lo:hi], rhs=rhs,
                                                     start=False, stop=last)
                            for ci, (y0, ny) in enumerate(grp):
                                nc.scalar.activation(out=ht[pp][:, 2 + y0:2 + y0 + ny, 2:2 + G],
                                                     in_=pss[ci][:, 0:ny, 0:G], func=AF.Relu,
                                                     bias=bias, scale=1.0)

                def emit_rt(s):
                    pp = s % 2
                    for grp in (NCH2[0:3], NCH2[3:6]):
                        pss = [psum.tile([97, 9, 51], FP32, tag="psA",
                                         name=f"rtps{s}_{y0}")
                               for (y0, ny) in grp]
                        for di in range(3):
                            for dj in range(3):
                                first = (di == 0 and dj == 0)
                                for ci, (y0, ny) in enumerate(grp):
                                    rhs = hA[pp][:, y0 + di:y0 + di + ny, dj:dj + 51]
                                    nc.tensor.matmul(out=pss[ci][:, 0:ny, :],
                                                     lhsT=wrtt[di][dj], rhs=rhs,
                                                     start=first, stop=False)
                        for di in range(3):
                            last = (di == 2)
                            for ci, (y0, ny) in enumerate(grp):
                                rhs = XBh[:, y0 + di:y0 + di + ny, 1:1 + 51]
                                nc.tensor.matmul(out=pss[ci][:, 0:ny, :],
                                                 lhsT=wrttB[di], rhs=rhs,
                                                 start=False, stop=last)
                        for ci, (y0, ny) in enumerate(grp):
                            nc.scalar.activation(out=e_s[:, y0:y0 + ny, 0:51],
                                                 in_=pss[ci][0:72, 0:ny, :], func=AF.Exp,
                                                 bias=0.0, scale=1.0)
                            nc.scalar.activation(out=r_s[:, y0:y0 + ny, 0:51],
                                                 in_=pss[ci][96:97, 0:ny, :], func=AF.Copy,
                                                 bias=0.0, scale=1.0)
                    # scatter e into sTk block layout: one DMA per row-block
                    # (72 src partitions -> one dst partition, (a,k) order
                    # transposed to (k,a) via two dst free dims).
                    sTf = sTk[:, :, :, :]
                    spstr = sTf.ap[0][0]        # A*9*RC
                    esf = e_s[:, :, :]
                    for rb in range(RB):
                        ln = RC if rb < 7 else 2 * PW   # rb 7 has rows 49,50
                        dst = bass.AP(tensor=sTf.tensor,
                                      offset=sTf.offset + (s * RB + rb) * spstr,
                                      ap=[[spstr, 1], [RC, 72], [1, ln]])
                        src = bass.AP(tensor=esf.tensor,
                                      offset=esf.offset + rb * RC,
                                      ap=[[PFLAT, 72], [1, ln]])
                        eng = nc.scalar if rb % 2 == 0 else nc.sync
                        eng.dma_start(out=dst, in_=src)
                    # reward + V seed (V0=0 makes step 1 trivially V1=R, so
                    # seed V's own rows with R and run one fewer step).
                    Rtf = Rt[:, :, :]
                    rpstr = Rtf.ap[0][0]        # RPB*PW
                    rsf = r_s[:, :, :]
                    Vtf = Vt[:, :, :]
                    vpstr = Vtf.ap[0][0]        # (RPB+2)*VC
                    nc.sync.dma_start(
                        out=bass.AP(tensor=Rtf.tensor,
                                    offset=Rtf.offset + (s * RB) * rpstr,
                                    ap=[[rpstr, 7], [1, RC]]),
                        in_=bass.AP(tensor=rsf.tensor, offset=rsf.offset,
                                    ap=[[PFLAT, 1], [RC, 7], [1, RC]]))
                    nc.sync.dma_start(
                        out=bass.AP(tensor=Rtf.tensor,
                                    offset=Rtf.offset + (s * RB + 7) * rpstr,
                                    ap=[[rpstr, 1], [1, 2 * PW]]),
                        in_=bass.AP(tensor=rsf.tensor, offset=rsf.offset + 7 * RC,
                                    ap=[[PFLAT, 1], [1, 2 * PW]]))
                    nc.scalar.dma_start(
                        out=bass.AP(tensor=Vtf.tensor,
                                    offset=Vtf.offset + (s * RB) * vpstr + VC + 1,
                                    ap=[[vpstr, 7], [VC, RPB], [1, PW]]),
                        in_=bass.AP(tensor=rsf.tensor, offset=rsf.offset,
                                    ap=[[PFLAT, 1], [RC, 7], [PW, RPB], [1, PW]]))
                    nc.scalar.dma_start(
                        out=bass.AP(tensor=Vtf.tensor,
                                    offset=Vtf.offset + (s * RB + 7) * vpstr + VC + 1,
                                    ap=[[vpstr, 1], [VC, 2], [1, PW]]),
                        in_=bass.AP(tensor=rsf.tensor, offset=rsf.offset + 7 * RC,
                                    ap=[[PFLAT, 1], [PW, 2], [1, PW]]))

                # software pipeline: PE stream is conv2(s), conv1(s+1),
                # rt(s-1). Delaying rt by one sample gives every X3 build a
                # full PE-iteration of slack, so the DMA builds never stall
                # the PE (the baseline lost ~430us to such gaps).
                emit_x1(0)
                emit_x1(1)
                emit_conv1(0)
                build_xb(XB2, h1B[0])
                # scan-tile zero-fill: 33us of DVE memset, emitted after the
                # conv head so it runs under conv1/conv2(0) instead of
                # delaying the first matmul's canvas setup. Only needed
                # before the first rt scatter.
                nc.vector.memset(sTk.rearrange("p a b c -> p (a b c)"), 0.0)
                nc.vector.memset(Vt.rearrange("p a b -> p (a b)"), 0.0)
                nc.vector.memset(Rt.rearrange("p a b -> p (a b)"), 0.0)
                for s in range(S):
                    emit_conv2(s)
                    if s + 1 < S:
                        emit_conv1(s + 1)
                        if s + 2 < S:
                            emit_x1(s + 2)
                        build_xb(XB2, h1B[(s + 1) % 2])
                    if s >= 1:
                        emit_rt(s - 1)
                    build_xb(XBh, hB[s % 2])
                    if debug_taps and s == 0:
                        nc.sync.dma_start(out=dbg_h1[:, :, :], in_=h1A[0][:, :, :])
                        nc.sync.dma_start(out=dbg_h[:, :, :], in_=hA[0][:, :, :])
                        nc.sync.dma_start(out=dbg_e[:, :, :], in_=e_s[:, :, :])
                emit_rt(S - 1)

            # ---------- softmax over k (block layout, k-major) ----------
            # denominators via a k-tree, +eps so the zero pad rows give a
            # finite reciprocal (0 * big = 0, not 0 * inf = NaN), then one
            # broadcast multiply normalizes all 9 k-planes.
            with tc.tile_pool(name="smx", bufs=1) as smx:
                t1 = smx.tile([128, A, 4, RC], FP16, tag="sx1")
                nc.vector.tensor_add(out=t1[:, :, :, :], in0=sTk[:, :, 0:4, :],
                                     in1=sTk[:, :, 4:8, :])
                nc.vector.tensor_add(out=t1[:, :, 0:2, :], in0=t1[:, :, 0:2, :],
                                     in1=t1[:, :, 2:4, :])
                nc.vector.tensor_add(out=t1[:, :, 0, :], in0=t1[:, :, 0, :],
                                     in1=t1[:, :, 1, :])
                nc.vector.tensor_add(out=t1[:, :, 0, :], in0=t1[:, :, 0, :],
                                     in1=sTk[:, :, 8, :])
                rec = smx.tile([128, A, RC], FP16, tag="sxr")
                lnD = smx.tile([128, A, RC], FP16, tag="sxln")
                epsb = smx.tile([128, 1], FP32, tag="sxeps")
                nc.vector.memset(epsb[:, :], 1e-3)
                nc.scalar.activation(out=lnD[:, :, :], in_=t1[:, :, 0, :],
                                     func=AF.Ln, bias=epsb[:, :], scale=1.0)
                nc.scalar.activation(out=rec[:, :, :], in_=lnD[:, :, :],
                                     func=AF.Exp, bias=0.0, scale=-1.0)
                sTf = sTk[:, :, :, :]
                spstr = sTf.ap[0][0]
                recf = rec[:, :, :]
                nc.vector.tensor_mul(
                    out=bass.AP(tensor=sTf.tensor, offset=sTf.offset,
                                ap=[[spstr, 128], [9 * RC, A], [RC, 9], [1, RC]]),
                    in0=bass.AP(tensor=sTf.tensor, offset=sTf.offset,
                                ap=[[spstr, 128], [9 * RC, A], [RC, 9], [1, RC]]),
                    in1=bass.AP(tensor=recf.tensor, offset=recf.offset,
                                ap=[[recf.ap[0][0], 128], [RC, A], [0, 9], [1, RC]]))

            # ---------- value-iteration scan (block layout) ----------
            # Per step: 2 halo DMAs (hidden under the di=1 multiply), 3
            # stride-0-broadcast multiplies (one per di), a 4-op k-sum tree,
            # a 3-op in-free-dim action max, and the reward add writing V's
            # own rows. No cross-partition max tree, no V replication.
            Vtf = Vt[:, :, :]
            vpstr = Vtf.ap[0][0]
            sTf = sTk[:, :, :, :]
            spstr = sTf.ap[0][0]

            # the DVE ISA allows at most 3 free dims, so the multiply is one
            # op per k = (di, dj): free dims (a broadcast, row, col)
            def vwin(di, dj):
                return bass.AP(tensor=Vtf.tensor,
                               offset=Vtf.offset + di * VC + dj,
                               ap=[[vpstr, 128], [0, A], [VC, RPB], [1, PW]])

            def kplane(full, k):
                return bass.AP(tensor=full.tensor, offset=full.offset + k * RC,
                               ap=[[full.ap[0][0], 128], [9 * RC, A],
                                   [PW, RPB], [1, PW]])

            with tc.tile_pool(name="scan", bufs=1) as scan, \
                 tc.tile_pool(name="scantail", bufs=2) as tail:
                P = scan.tile([128, A, 9, RC], FP16, tag="P")
                Rtf = Rt[:, :, :]
                rpstr = Rtf.ap[0][0]
                for t in range(1, KST):
                    # halo exchange from the V of the previous step: slot 0
                    # <- previous partition's last own row, slot 8 <- next
                    # partition's first own row. Runs on both HWDGE queues
                    # under the (halo-free) di=1 multiply.
                    nc.sync.dma_start(
                        out=bass.AP(tensor=Vtf.tensor,
                                    offset=Vtf.offset + vpstr,
                                    ap=[[vpstr, 127], [1, VC]]),
                        in_=bass.AP(tensor=Vtf.tensor,
                                    offset=Vtf.offset + 7 * VC,
                                    ap=[[vpstr, 127], [1, VC]]))
                    nc.scalar.dma_start(
                        out=bass.AP(tensor=Vtf.tensor,
                                    offset=Vtf.offset + 8 * VC,
                                    ap=[[vpstr, 127], [1, VC]]),
                        in_=bass.AP(tensor=Vtf.tensor,
                                    offset=Vtf.offset + vpstr + VC,
                                    ap=[[vpstr, 127], [1, VC]]))
                    Pf = P[:, :, :, :]
                    for di in (1, 0, 2):
                        for dj in range(3):
                            k = di * 3 + dj
                            nc.vector.tensor_mul(out=kplane(Pf, k),
                                                 in0=kplane(sTf, k),
                                                 in1=vwin(di, dj))
                    # k-sum tree (4 ops)
                    nc.vector.tensor_add(out=P[:, :, 0:4, :], in0=P[:, :, 0:4, :],
                                         in1=P[:, :, 4:8, :])
                    nc.vector.tensor_add(out=P[:, :, 0:2, :], in0=P[:, :, 0:2, :],
                                         in1=P[:, :, 2:4, :])
                    nc.vector.tensor_add(out=P[:, :, 0, :], in0=P[:, :, 0, :],
                                         in1=P[:, :, 1, :])
                    nc.vector.tensor_add(out=P[:, :, 0, :], in0=P[:, :, 0, :],
                                         in1=P[:, :, 8, :])
                    if t == KST - 1:
                        nc.vector.tensor_add(
                            out=qL[:, :, :], in0=P[:, :, 0, :],
                            in1=bass.AP(tensor=Rtf.tensor, offset=Rtf.offset,
                                        ap=[[rpstr, 128], [0, A], [1, RC]]))
                        break
                    # action max in the free dim (in place on P[:, :, 0])
                    nc.vector.tensor_max(out=P[:, 0:4, 0, :], in0=P[:, 0:4, 0, :],
                                         in1=P[:, 4:8, 0, :])
                    nc.vector.tensor_max(out=P[:, 0:2, 0, :], in0=P[:, 0:2, 0, :],
                                         in1=P[:, 2:4, 0, :])
                    vm = tail.tile([128, RPB, PW], FP16, tag="vm")
                    nc.vector.tensor_max(out=vm.rearrange("p a b -> p (a b)"),
                                         in0=P[:, 0, 0, :], in1=P[:, 1, 0, :])
                    # reward add writes V's own rows (pad rows stay 0: their
                    # sT and R are 0, so max over a gives 0 + 0)
                    nc.vector.tensor_add(out=Vt[:, 1:1 + RPB, 1:1 + PW],
                                         in0=vm[:, :, :], in1=Rt[:, :, :])

            # ---------- per-pixel MLP ----------
            with tc.tile_pool(name="mlp", bufs=4) as mlp, \
                 tc.tile_pool(name="mpsum", bufs=3, space="PSUM") as mpsum:
                qLf = qL[:, :, :]
                qpstr = qLf.ap[0][0]
                # gather all samples' q into one [a, s, 56-row] staging tile
                # with 32 wide DMAs (8 actions x 4 sample groups), all on the
                # sync queue: the scalar queue stays free for the MLP's
                # activations (a dma_start costs ~730ns of queue-engine time,
                # so per-sample-per-action gathers starve the Act engine).
                qAll = mlp.tile([A, S, RB * RPB, PW], FP16, tag="qAll", bufs=1)
                qAf = qAll[:, :, :, :]
                qapstr = qAf.ap[0][0]
                for g in range(4):
                    for a in range(A):
                        dst = bass.AP(tensor=qAf.tensor,
                                      offset=qAf.offset + a * qapstr
                                      + g * 4 * RB * RC,
                                      ap=[[qapstr, 1], [RC, 32], [1, RC]])
                        srcp = bass.AP(tensor=qLf.tensor,
                                      offset=qLf.offset + g * 32 * qpstr + a * RC,
                                      ap=[[qpstr, 32], [1, RC]])
                        eng = (nc.scalar if (g == 0 and a % 2 == 1)
                               else nc.sync)
                        eng.dma_start(out=dst, in_=srcp)
                for s in range(S):
                    midA = mlp.tile([128, G, G], FP16, tag="midA", bufs=3)
                    midB = mlp.tile([22, G, G], FP16, tag="midB", bufs=3)
                    # half-outer order: one LDWEIGHTS per co-half instead of
                    # alternating weights every matmul
                    for (y0, ny) in MLPN:
                        rhs = qAll[:, s, y0:y0 + ny, 0:G]
                        p1 = mpsum.tile([128, 10, G], FP32, tag="m1")
                        nc.tensor.matmul(out=p1[:, 0:ny, :],
                                         lhsT=wa1t[:, 0:128], rhs=rhs,
                                         start=True, stop=True)
                        nc.scalar.activation(out=midA[:, y0:y0 + ny, :],
                                             in_=p1[:, 0:ny, :], func=AF.Relu,
                                             bias=ba1A, scale=1.0)
                    for (y0, ny) in MLPN:
                        rhs = qAll[:, s, y0:y0 + ny, 0:G]
                        p2 = mpsum.tile([22, 10, G], FP32, tag="m2", bufs=2)
                        nc.tensor.matmul(out=p2[:, 0:ny, :],
                                         lhsT=wa1t[:, 128:150], rhs=rhs,
                                         start=True, stop=True)
                        nc.vector.tensor_scalar(
                            out=midB[:, y0:y0 + ny, :],
                            in0=p2[:, 0:ny, :], scalar1=ba1B, scalar2=0.0,
                            op0=mybir.AluOpType.add, op1=mybir.AluOpType.max)
                    ost = mlp.tile([A, G, G], FP16, tag="ost", bufs=3)
                    for ci, (y0, ny) in enumerate(MLPN):
                        p3 = mpsum.tile([A, 10, G], FP32, tag="m3")
                        nc.tensor.matmul(out=p3[:, 0:ny, :], lhsT=wa2A,
                                         rhs=midA[:, y0:y0 + ny, :],
                                         start=True, stop=False)
                        nc.tensor.matmul(out=p3[:, 0:ny, :], lhsT=wa2B,
                                         rhs=midB[:, y0:y0 + ny, :],
                                         start=False, stop=True)
                        # L2 drains on the DVE: moving any to the Act
                        # engine serializes them behind the next sample's
                        # midA relus on the in-order Act queue (measured +4us)
                        nc.vector.tensor_scalar(
                            out=ost[:, y0:y0 + ny, :],
                            in0=p3[:, 0:ny, :], scalar1=ba2t, scalar2=None,
                            op0=mybir.AluOpType.add)
                    if s == S - 1:
                        nc.scalar.dma_start(out=out[s, 0:4, :, :],
                                            in_=ost[0:4, :, :])
                        nc.sync.dma_start(out=out[s, 4:8, :, :],
                                          in_=ost[4:8, :, :])
                    else:
                        eng = nc.scalar if s % 2 == 0 else nc.sync
                        eng.dma_start(
                            out=out[s, :, :, :],
                            in_=ost[:, :, :])

    _split_multi_waits(nc)
    return nc


def _prep_weights(inputs):
    f32 = lambda x: np.asarray(x, dtype=np.float32)
    h1_w = f32(inputs["h1_w"]); h1_b = f32(inputs["h1_b"])
    h2_w = f32(inputs["h2_w"]); h2_b = f32(inputs["h2_b"])
    r_w = f32(inputs["r_w"]); t_w = f32(inputs["t_w"])
    a1_w = f32(inputs["a1_w"]); a1_b = f32(inputs["a1_b"])
    a2_w = f32(inputs["a2_w"]); a2_b = f32(inputs["a2_b"])

    # conv1: k = ci*9 + di*3 + dj
    w1 = h1_w.transpose(1, 2, 3, 0).reshape(18, HID).astype(np.float16)
    # conv2 / rt: A tiles [di, dj, ci<128, co]; B tiles [di, (dj, ci>=128), co]
    w2f = np.ascontiguousarray(
        h2_w.transpose(2, 3, 1, 0), dtype=np.float32)   # [di, dj, ci, co]
    w2A = w2f[:, :, 0:128, :].astype(np.float16)
    w2B = np.ascontiguousarray(
        w2f[:, :, 128:150, :]).reshape(3, 66, HID).astype(np.float16)
    wrt_full = np.zeros((3, 3, HID, 97), np.float32)
    wrt_full[:, :, :, 0:72] = t_w.transpose(2, 3, 1, 0)  # [di, dj, ci, m]
    wrt_full[:, :, :, 96:97] = r_w.transpose(2, 3, 1, 0)
    wrtA = wrt_full[:, :, 0:128, :].astype(np.float16)
    wrtB = np.ascontiguousarray(
        wrt_full[:, :, 128:150, :]).reshape(3, 66, 97).astype(np.float16)
    return {
        "w1": w1, "b1": h1_b.reshape(HID, 1),
        "w2a": w2A, "w2b": w2B, "b2": h2_b.reshape(HID, 1),
        "wrta": wrtA, "wrtb": wrtB,
        "wa1": a1_w.T.astype(np.float16).copy(),      # [8, 150]
        "ba1": a1_b.reshape(HID, 1),
        "wa2": a2_w.T.astype(np.float16).copy(),      # [150, 8]
        "ba2": a2_b.reshape(A, 1),
    }


_CACHE = {}


def _get_program():
    if "nc" not in _CACHE:
        _CACHE["nc"] = build_program()
    return _CACHE["nc"]


def kernel(**inputs):
    nc = _get_program()
    grid = np.asarray(inputs["grid"], dtype=np.float32)
    wts = _prep_weights(inputs)
    in_maps = []
    for c in range(8):
        m = {"grid16": grid[c * S:(c + 1) * S].astype(np.float16)}
        m.update(wts)
        in_maps.append(m)
    res = run_bass_kernel_spmd(nc, in_maps, core_ids=list(range(8)))
    outp = np.concatenate([res.results[c]["o"] for c in range(8)], axis=0)
    return outp.astype(np.float32)


def run_traced(inputs, tmpdir):
    """Like kernel() but with NTFF profiling; returns (output, exec_time_ns)."""
    import ctypes, contextlib
    sys.path.insert(0, "/root/.axon_site/trn_agent_boot")
    import trn_boot
    hook = trn_boot._ntff_profile_via_ctypes("/opt/axon/libaxon_pjrt.so")
    mod = types.ModuleType("antenv.axon_hooks")
    mod.get_axon_ntff_profile_hook = lambda: hook
    sys.modules["antenv.axon_hooks"] = mod

    nc = _get_program()
    grid = np.asarray(inputs["grid"], dtype=np.float32)
    wts = _prep_weights(inputs)
    in_maps = []
    for c in range(8):
        m = {"grid16": grid[c * S:(c + 1) * S].astype(np.float16)}
        m.update(wts)
        in_maps.append(m)
    res = run_bass_kernel_spmd(nc, in_maps, core_ids=list(range(8)),
                               trace=True, tmpdir=tmpdir)
    outp = np.concatenate([res.results[c]["o"] for c in range(8)], axis=0)
    return outp.astype(np.float32), res.exec_time_ns

